# revision 3
# baseline (speedup 1.0000x reference)
"""nn_ASTGPool Trainium2 kernel: per-graph top-k pooling over 8 NeuronCores.

Sharding (per spec hint): graphs are assigned to cores by graph id (8 graphs
per core); each graph's nodes and edges are colocated on its core. The host
buckets/permutes the edge list by (graph, src) and pads each graph to a fixed
17408 edges; all counting/ranking/top-k/gather/masking runs on device. The
host inverse-permutes the edge outputs back to the original edge order.

Device algorithm per core (see dev/kern_core.py history):
  1. degree histogram via one-hot (hi/lo 32x32) matmuls accumulated in PSUM
  2. exact jax.lax.top_k rank via counting (prefix scans + small PE matmuls):
     rank(i) = #{deg > deg_i} + #{j < i: deg_j == deg_i}
  3. perm/score scatter via indirect DMA (drop unselected via bounds check)
  4. x row gather via indirect DMA
  5. src-side edge remap via run-length expansion (scatter at cumsum offsets
     + prefix-max scan); dst-side via GPSIMD ap_gather of the node_map table
  6. edge mask + global id remap, all fused elementwise
"""
import sys

for p in ("/opt/trn_rl_repo",):
    if p not in sys.path:
        sys.path.insert(0, p)

import numpy as np
import ml_dtypes

from concourse import bass, mybir, bass_utils
import concourse.bacc as bacc
import concourse.tile as tile
from concourse.bass import IndirectOffsetOnAxis

F32, BF16, I32, I16, U8 = (mybir.dt.float32, mybir.dt.bfloat16,
                           mybir.dt.int32, mybir.dt.int16, mybir.dt.uint8)
OP = mybir.AluOpType

G, N, E, F = 64, 1024, 1_048_576, 256
K = 512
NC = 8
GPC = G // NC              # graphs per core
NPC = GPC * N              # nodes per core
EG = 17408                 # padded edges per graph (16 * 1088)
EC = GPC * EG              # padded edges per core
COLS = EC // 128           # 1088
VMAX = 64                  # degree value bins (max degree must be < VMAX)


# ----------------------------------------------------------------- device --
def build_kernel(tc, outs, ins):
    nc = tc.nc
    cp = tc.alloc_tile_pool(name="const", bufs=1)
    psp = tc.alloc_tile_pool(name="psum", bufs=1, space="PSUM")
    work = tc.alloc_tile_pool(name="work", bufs=1)

    nm_dram = nc.dram_tensor("nm_scratch", (NPC,), I32, kind="Internal").ap()
    enc_dram = nc.dram_tensor("enc_scratch", (2 * EC,), F32,
                              kind="Internal").ap()
    rd_dram = nc.dram_tensor("rd_scratch", (EC,), I32, kind="Internal").ap()

    # ---------------- constants ----------------
    io32i = cp.tile([128, 32], I32)
    nc.gpsimd.iota(io32i[:], pattern=[[1, 32]], base=0, channel_multiplier=0)
    io32b = cp.tile([128, 32], BF16)
    nc.vector.tensor_copy(io32b[:], io32i[:])
    ioVi = cp.tile([128, VMAX], I32)
    nc.gpsimd.iota(ioVi[:], pattern=[[1, VMAX]], base=0, channel_multiplier=0)
    ioVb = cp.tile([128, VMAX], BF16)
    nc.vector.tensor_copy(ioVb[:], ioVi[:])

    pidx = cp.tile([128, 1], I32)
    nc.gpsimd.iota(pidx[:], pattern=[[0, 1]], base=0, channel_multiplier=1)
    pdiv = cp.tile([128, 1], I32)      # p // 32
    nc.vector.tensor_scalar(pdiv[:], pidx[:], 5, None,
                            op0=OP.logical_shift_right)
    pmod = cp.tile([128, 1], I32)      # p % 32 = p - ((p>>5)<<5)
    nc.vector.tensor_scalar(pmod[:], pdiv[:], 5, None,
                            op0=OP.logical_shift_left)
    nc.vector.tensor_tensor(pmod[:], pidx[:], pmod[:], op=OP.subtract)
    pmodb = cp.tile([128, 1], BF16)
    nc.vector.tensor_copy(pmodb[:], pmod[:])
    io1088 = cp.tile([128, 1], I32)    # 1088 * p
    nc.gpsimd.iota(io1088[:], pattern=[[0, 1]], base=0,
                   channel_multiplier=1088)
    io1088f = cp.tile([128, 1], F32)
    nc.vector.tensor_copy(io1088f[:], io1088[:])

    L32 = cp.tile([128, 32], F32)      # [p, m] = 1 if p%32 < m
    nc.vector.tensor_tensor(L32[:], pmodb[:].to_broadcast([128, 32]),
                            io32b[:], op=OP.is_lt)
    E0 = cp.tile([128, 32], F32)       # [p, m] = 1 if p%32 == 0
    nc.vector.tensor_scalar(E0[:], pmodb[:].to_broadcast([128, 32]), 0.0,
                            None, op0=OP.is_equal)
    ones32 = cp.tile([128, 32], F32)
    nc.vector.memset(ones32[:], 1.0)

    glK = cp.tile([128, 2], F32)       # (t*4 + p//32) * 512
    glKi = cp.tile([128, 2], I32)
    nc.vector.tensor_scalar(glKi[:, 0:1], pdiv[:], 9, None,
                            op0=OP.logical_shift_left)
    nc.vector.tensor_scalar(glKi[:, 1:2], glKi[:, 0:1], 2048, None,
                            op0=OP.add)
    nc.vector.tensor_copy(glK[:], glKi[:])

    nid = cp.tile([128, 2, 32], I32)   # local node id = 32p + 4096t + lo
    nc.gpsimd.iota(nid[:], pattern=[[4096, 2], [1, 32]], base=0,
                   channel_multiplier=32)

    bsb = cp.tile([1, 2], I32)
    nc.sync.dma_start(bsb[:], ins["bases"][:])
    bbc = cp.tile([128, 2], I32)
    nc.gpsimd.partition_broadcast(bbc[:], bsb[:])
    bbcf = cp.tile([128, 2], F32)
    nc.vector.tensor_copy(bbcf[:], bbc[:])

    maskD = cp.tile([128, 2, 32], BF16)
    nc.vector.memset(maskD[:], 1.0)
    nc.vector.memset(maskD[:, :, 0:1], 0.0)

    # ---------------- inputs ----------------
    hi_sb = work.tile([128, COLS], BF16, tag="hi")
    lo_sb = work.tile([128, COLS], BF16, tag="lo")
    dstW_sb = work.tile([128, COLS], I16, tag="dstw")
    nc.sync.dma_start(hi_sb[:], ins["hiH"][:])
    nc.sync.dma_start(lo_sb[:], ins["loH"][:])
    nc.sync.dma_start(dstW_sb[:], ins["dstW"][:])

    # ---------------- phase 1: degree histogram ----------------
    deg_ps = psp.tile([128, 2, 32], F32, tag="deg")
    hp = tc.alloc_tile_pool(name="hist", bufs=2)
    for blk in range(4):
        c0 = blk * 272
        A = hp.tile([128, 272, 32], BF16, tag="A")
        B = hp.tile([128, 272, 32], BF16, tag="B")
        nc.vector.tensor_tensor(
            A[:], hi_sb[:, c0:c0 + 272].to_broadcast([128, 272, 32]),
            io32b[:][:, None, :].to_broadcast([128, 272, 32]),
            op=OP.is_equal)
        nc.vector.tensor_tensor(
            B[:], lo_sb[:, c0:c0 + 272].to_broadcast([128, 272, 32]),
            io32b[:][:, None, :].to_broadcast([128, 272, 32]),
            op=OP.is_equal)
        for g2 in range(2):
            g = blk * 2 + g2
            q, t = g % 4, g // 4
            for c in range(136):
                col = g2 * 136 + c
                nc.tensor.matmul(
                    deg_ps[32 * q:32 * q + 32, t, :],
                    lhsT=A[:, col, :], rhs=B[:, col, :],
                    start=(c == 0), stop=(c == 135),
                    tile_position=(0, 32 * q))
    hp.release()

    deg = work.tile([128, 2, 32], F32, tag="deg_sb")
    nc.vector.tensor_copy(deg[:], deg_ps[:])
    degb = work.tile([128, 2, 32], BF16, tag="degb")
    nc.vector.tensor_copy(degb[:], deg_ps[:])

    # ---------------- phase 2: rank ----------------
    rp = tc.alloc_tile_pool(name="rankp", bufs=1)
    maskE = rp.tile([128, 2, VMAX, 32], BF16, tag="maskE")
    nc.vector.memset(maskE[:], 1.0)
    nc.vector.memset(maskE[:, :, :, 0:1], 0.0)
    E2 = rp.tile([128, 2, VMAX, 32], BF16, tag="E2")   # [p, t, v, lo]
    nc.vector.tensor_tensor(
        E2[:], degb[:][:, :, None, :].to_broadcast([128, 2, VMAX, 32]),
        ioVb[:][:, None, :, None].to_broadcast([128, 2, VMAX, 32]),
        op=OP.is_equal)
    Esh = rp.tile([128, 2, VMAX, 32], BF16, tag="Esh")
    nc.vector.memset(Esh[:, :, :, 0:1], 0.0)
    nc.vector.tensor_copy(Esh[:, :, :, 1:], E2[:, :, :, 0:31])
    Pex = rp.tile([128, 2, VMAX, 32], BF16, tag="Pex")
    nc.vector.tensor_tensor_scan(
        Pex[:].rearrange("p t v l -> p (t v l)"),
        Esh[:].rearrange("p t v l -> p (t v l)"),
        maskE[:].rearrange("p t v l -> p (t v l)"),
        initial=0.0, op0=OP.add, op1=OP.mult)
    S = work.tile([128, 2, VMAX], F32, tag="S")
    nc.vector.tensor_tensor(S[:], Pex[:, :, :, 31], E2[:, :, :, 31],
                            op=OP.add)

    hist_ps = psp.tile([128, 2, VMAX], F32, tag="hist")
    for g in range(8):
        q, t = g % 4, g // 4
        r = slice(32 * q, 32 * q + 32)
        nc.tensor.matmul(hist_ps[r, t, :], lhsT=ones32[r, :], rhs=S[r, t, :],
                         start=True, stop=True,
                         tile_position=(32 * q, 32 * q))
    hist_sb = work.tile([128, 2, VMAX], F32, tag="hist_sb")
    nc.vector.tensor_copy(hist_sb[:], hist_ps[:])
    Ta = work.tile([128, 2, VMAX], F32, tag="Ta")
    Tb = work.tile([128, 2, VMAX], F32, tag="Tb")
    nc.vector.memset(Ta[:, :, VMAX - 1:VMAX], 0.0)
    nc.vector.tensor_copy(Ta[:, :, 0:VMAX - 1], hist_sb[:, :, 1:VMAX])
    cur, oth = Ta, Tb
    for sh in (1, 2, 4, 8, 16, 32):
        nc.vector.tensor_copy(oth[:], cur[:])
        nc.vector.tensor_tensor(oth[:, :, 0:VMAX - sh],
                                cur[:, :, 0:VMAX - sh],
                                cur[:, :, sh:VMAX], op=OP.add)
        cur, oth = oth, cur
    Tsuf = cur

    carry_ps = psp.tile([128, 2, VMAX], F32, tag="carry")
    for g in range(8):
        q, t = g % 4, g // 4
        r = slice(32 * q, 32 * q + 32)
        nc.tensor.matmul(carry_ps[r, t, :], lhsT=L32[r, :], rhs=S[r, t, :],
                         start=True, stop=False,
                         tile_position=(32 * q, 32 * q))
        nc.tensor.matmul(carry_ps[r, t, :], lhsT=E0[r, :], rhs=Tsuf[r, t, :],
                         start=False, stop=True,
                         tile_position=(32 * q, 32 * q))

    QA = rp.tile([128, 2, VMAX, 32], F32, tag="QA")
    nc.vector.tensor_tensor(
        QA[:], Pex[:], carry_ps[:].to_broadcast([128, 2, VMAX, 32]),
        op=OP.add)
    QB = rp.tile([128, 2, 32, VMAX], F32, tag="QB")    # [p, t, lo, v]
    nc.vector.tensor_tensor(QB[:].rearrange("p t l v -> p t v l"), QA[:],
                            E2[:], op=OP.mult)
    rank = work.tile([128, 2, 32], F32, tag="rank")
    nc.vector.tensor_reduce(rank[:], QB[:], axis=mybir.AxisListType.X,
                            op=OP.add)
    rp.release()

    # ---------------- phase 3: node_map / scatter perm+score --------------
    msel = work.tile([128, 2, 32], F32, tag="msel")
    nc.vector.tensor_scalar(msel[:], rank[:], float(K), None, op0=OP.is_lt)
    t1 = work.tile([128, 2, 32], F32, tag="t1")
    nc.vector.tensor_tensor(t1[:], rank[:],
                            glK[:].to_broadcast([128, 2, 32]), op=OP.add)
    nm_f = work.tile([128, 2, 32], F32, tag="nm_f")
    nc.vector.tensor_scalar(nm_f[:], t1[:], 1.0, None, op0=OP.add)
    nc.vector.tensor_tensor(nm_f[:], nm_f[:], msel[:], op=OP.mult)
    nc.vector.tensor_scalar(nm_f[:], nm_f[:], -1.0, None, op0=OP.add)
    offs_f = work.tile([128, 2, 32], F32, tag="offs_f")
    nc.vector.tensor_scalar(offs_f[:], msel[:], -100000.0, 100000.0,
                            op0=OP.mult, op1=OP.add)
    nc.vector.tensor_tensor(offs_f[:], offs_f[:], t1[:], op=OP.add)
    offs_i = work.tile([128, 2, 32], I32, tag="offs_i")
    nc.vector.tensor_copy(offs_i[:], offs_f[:])

    pay = work.tile([128, 2, 32, 2], I32, tag="pay")
    nid_f = work.tile([128, 2, 32], F32, tag="nid_f")
    nc.vector.tensor_copy(nid_f[:], nid[:])
    nc.vector.tensor_scalar(nid_f[:], nid_f[:], bbcf[:, 0:1], None,
                            op0=OP.add)
    nidg = work.tile([128, 2, 32], I32, tag="nidg")
    nc.vector.tensor_copy(nidg[:], nid_f[:])
    nc.vector.tensor_copy(pay[:, :, :, 0], nidg[:])
    nc.vector.tensor_copy(pay[:, :, :, 1], deg[:].bitcast(I32))
    nc.gpsimd.indirect_dma_start(
        out=outs["permscore"][:], out_offset=IndirectOffsetOnAxis(
            ap=offs_i[:].rearrange("p t l -> p (t l)"), axis=0),
        in_=pay[:].rearrange("p t l x -> p (t l) x"), in_offset=None,
        bounds_check=GPC * K - 1, oob_is_err=False)

    nm_i = work.tile([128, 2, 32], I32, tag="nm_i")
    nc.vector.tensor_copy(nm_i[:], nm_f[:])
    nc.sync.dma_start(
        nm_dram.rearrange("(t q h l) -> (q h) t l", t=2, q=4, h=32, l=32),
        nm_i[:])
    ep = tc.alloc_tile_pool(name="edgep", bufs=1)
    table = ep.tile([128, NPC], I32, tag="table")
    nc.sync.dma_start(table[:], nm_dram[None, :].to_broadcast([128, NPC]))

    # ---------------- phase 4: start offsets + enc scatter ----------------
    degsh = work.tile([128, 2, 32], F32, tag="degsh")
    nc.vector.memset(degsh[:, :, 0:1], 0.0)
    nc.vector.tensor_copy(degsh[:, :, 1:], deg[:, :, 0:31])
    strel = work.tile([128, 2, 32], F32, tag="strel")
    nc.vector.tensor_tensor_scan(
        strel[:].rearrange("p t l -> p (t l)"),
        degsh[:].rearrange("p t l -> p (t l)"),
        maskD[:].rearrange("p t l -> p (t l)"),
        initial=0.0, op0=OP.add, op1=OP.mult)
    Stot = work.tile([128, 2], F32, tag="Stot")
    nc.vector.tensor_tensor(Stot[:], strel[:, :, 31], deg[:, :, 31],
                            op=OP.add)
    carry2 = psp.tile([128, 2, 1], F32, tag="carry2")
    for g in range(8):
        q, t = g % 4, g // 4
        r = slice(32 * q, 32 * q + 32)
        nc.tensor.matmul(carry2[r, t, :], lhsT=L32[r, :],
                         rhs=Stot[r, t][:, None], start=True, stop=True,
                         tile_position=(32 * q, 32 * q))
    start_f = work.tile([128, 2, 32], F32, tag="start_f")
    nc.vector.tensor_tensor(start_f[:], strel[:],
                            carry2[:].to_broadcast([128, 2, 32]), op=OP.add)
    gbase = work.tile([128, 2], F32, tag="gbase")
    nc.vector.tensor_scalar(gbase[:], glK[:], 34.0, None, op0=OP.mult)
    nc.vector.tensor_tensor(start_f[:], start_f[:],
                            gbase[:].to_broadcast([128, 2, 32]), op=OP.add)
    start_i = work.tile([128, 2, 32], I32, tag="start_i")
    nc.vector.tensor_copy(start_i[:], start_f[:])

    pay2 = work.tile([128, 2, 32, 2], F32, tag="pay2")
    nc.vector.tensor_copy(pay2[:, :, :, 0], start_f[:])
    nc.vector.tensor_scalar(pay2[:, :, :, 1], nm_f[:], 2.0, None, op0=OP.add)
    offst = work.tile([128, 2, 32], F32, tag="offst")
    nc.vector.tensor_scalar(offst[:], deg[:], 0.0, None, op0=OP.is_equal)
    nc.vector.tensor_scalar(offst[:], offst[:], 1000000.0, None, op0=OP.mult)
    nc.vector.tensor_tensor(offst[:], offst[:], start_f[:], op=OP.add)
    offst_i = work.tile([128, 2, 32], I32, tag="offst_i")
    nc.vector.tensor_copy(offst_i[:], offst[:])

    zed = ep.tile([128, 2 * COLS], F32, tag="zed")
    nc.vector.memset(zed[:], 0.0)
    nc.sync.dma_start(enc_dram.rearrange("(p c) -> p c", p=128), zed[:])
    nc.gpsimd.indirect_dma_start(
        out=enc_dram.rearrange("(n x) -> n x", x=2),
        out_offset=IndirectOffsetOnAxis(
            ap=offst_i[:].rearrange("p t l -> p (t l)"), axis=0),
        in_=pay2[:].rearrange("p t l x -> p (t l) x"),
        in_offset=None, bounds_check=EC - 1, oob_is_err=False)

    # ---------------- phase 5: src-side scan ----------------
    enc_sb = ep.tile([128, COLS, 2], F32, tag="enc_sb")
    nc.sync.dma_start(enc_sb[:],
                      enc_dram.rearrange("(p c x) -> p c x", p=128, x=2))
    encc = ep.tile([128, COLS], F32, tag="encc")
    nc.vector.tensor_scalar(encc[:], enc_sb[:, :, 0], io1088f[:], None,
                            op0=OP.subtract)
    nc.vector.tensor_scalar(encc[:], encc[:], 8192.0, None, op0=OP.mult)
    nc.vector.tensor_tensor(encc[:], encc[:], enc_sb[:, :, 1], op=OP.add)
    zedb = ep.tile([128, COLS], BF16, tag="zedb")
    nc.vector.memset(zedb[:], 0.0)
    scan = ep.tile([128, COLS], F32, tag="scan")
    nc.vector.tensor_tensor_scan(scan[:], encc[:], zedb[:],
                                 initial=0.0, op0=OP.max, op1=OP.bypass)
    pe = ep.tile([128, 1], F32, tag="pe")
    nc.vector.memset(pe[0:1, :], 0.0)
    nc.sync.dma_start(pe[1:128, :], scan[0:127, COLS - 1:COLS])
    fixed = ep.tile([128, COLS], F32, tag="fixed")
    pred = ep.tile([128, COLS], U8, tag="pred")
    nc.vector.tensor_scalar(pred[:], scan[:], 0.0, None, op0=OP.is_le)
    nc.vector.tensor_copy(fixed[:], scan[:])
    nc.vector.copy_predicated(fixed[:], pred[:],
                              pe[:].to_broadcast([128, COLS]))
    rs = ep.tile([128, COLS], I32, tag="rs")
    nc.vector.tensor_copy(rs[:], fixed[:])
    nc.vector.tensor_scalar(rs[:], rs[:], 8191, None, op0=OP.bitwise_and)
    nc.vector.tensor_scalar(rs[:], rs[:], -2, None, op0=OP.add)

    # ---------------- phase 6: dst gather + finalize ----------------
    for h in range(2):
        gat = ep.tile([128, EG // 2, 1], I32, tag="gat")
        nc.gpsimd.ap_gather(
            gat[:], table[:][:, :, None],
            dstW_sb[:, 544 * h:544 * (h + 1)],
            channels=128, num_elems=NPC, d=1, num_idxs=EG // 2)
        nc.sync.dma_start(
            rd_dram.rearrange("(g x j) -> g x j", g=8, x=2)[:, h, :],
            gat[0:128:16, :, 0])
    rd = ep.tile([128, COLS], I32, tag="rd")
    nc.sync.dma_start(rd[:], rd_dram.rearrange("(p c) -> p c", p=128))

    m8 = ep.tile([128, COLS], I32, tag="m8")
    nc.vector.tensor_scalar(m8[:], rs[:], 0, None, op0=OP.is_ge)
    m8b = ep.tile([128, COLS], I32, tag="m8b")
    nc.vector.tensor_scalar(m8b[:], rd[:], 0, None, op0=OP.is_ge)
    nc.vector.tensor_tensor(m8[:], m8[:], m8b[:], op=OP.mult)

    rsf = ep.tile([128, COLS], F32, tag="rsf")
    nc.vector.tensor_copy(rsf[:], rs[:])
    nc.vector.tensor_scalar(rsf[:], rsf[:], bbcf[:, 1:2], 1.0, op0=OP.add,
                            op1=OP.add)
    nc.vector.tensor_tensor(rsf[:], rsf[:], m8[:], op=OP.mult)
    nc.vector.tensor_scalar(rsf[:], rsf[:], -1.0, None, op0=OP.add)
    rsg = ep.tile([128, COLS], I32, tag="rsg")
    nc.vector.tensor_copy(rsg[:], rsf[:])
    rdf = ep.tile([128, COLS], F32, tag="rdf")
    nc.vector.tensor_copy(rdf[:], rd[:])
    nc.vector.tensor_scalar(rdf[:], rdf[:], bbcf[:, 1:2], 1.0, op0=OP.add,
                            op1=OP.add)
    nc.vector.tensor_tensor(rdf[:], rdf[:], m8[:], op=OP.mult)
    nc.vector.tensor_scalar(rdf[:], rdf[:], -1.0, None, op0=OP.add)
    rdg = ep.tile([128, COLS], I32, tag="rdg")
    nc.vector.tensor_copy(rdg[:], rdf[:])
    mu8 = ep.tile([128, COLS], U8, tag="mu8")
    nc.vector.tensor_copy(mu8[:], m8[:])

    nc.sync.dma_start(outs["rs"].rearrange("(p c) -> p c", p=128), rsg[:])
    nc.sync.dma_start(outs["rd"].rearrange("(p c) -> p c", p=128), rdg[:])
    nc.sync.dma_start(outs["mask"].rearrange("(p c) -> p c", p=128), mu8[:])

    # ---------------- phase 7: x gather ----------------
    po = ep.tile([128, 32], I32, tag="po")
    nc.sync.dma_start(
        po[:],
        outs["permscore"].rearrange("(j p) x -> p j x", p=128)[:, :, 0])
    po_f = ep.tile([128, 32], F32, tag="po_f")
    nc.vector.tensor_copy(po_f[:], po[:])
    nc.vector.tensor_scalar(po_f[:], po_f[:], bbcf[:, 0:1], None,
                            op0=OP.subtract)
    nc.vector.tensor_copy(po[:], po_f[:])
    xg = ep.tile([128, 32, 256], F32, tag="xg")
    nc.gpsimd.indirect_dma_start(
        out=xg[:], out_offset=None, in_=ins["x"][:],
        in_offset=IndirectOffsetOnAxis(ap=po[:], axis=0))
    nc.sync.dma_start(outs["xf"].rearrange("(j p) f -> p j f", p=128), xg[:])

    ep.release()
    work.release()
    psp.release()
    cp.release()


# ------------------------------------------------------------------- host --
def host_shard(x, edge_index, batch):
    src, dst = np.asarray(edge_index[0]), np.asarray(edge_index[1])
    order = np.argsort(src, kind="stable")
    g_sorted = src[order] >> 10
    bounds = np.searchsorted(g_sorted, np.arange(G + 1))
    in_maps, eorders = [], []
    for c in range(NC):
        src_pad = np.full((GPC, EG), N, np.int32)
        dst_pad = np.zeros((GPC, EG), np.int32)
        eorder = np.full((GPC, EG), -1, np.int64)
        for gl in range(GPC):
            g = c * GPC + gl
            sel = order[bounds[g]:bounds[g + 1]]
            cnt = sel.size
            assert cnt <= EG, f"graph {g} has {cnt} edges > pad {EG}"
            src_pad[gl, :cnt] = src[sel] - g * N
            dst_pad[gl, :cnt] = dst[sel] - c * NPC
            eorder[gl, :cnt] = sel
        srcH = src_pad.reshape(GPC, COLS // 8, 128).transpose(2, 0, 1) \
            .reshape(128, COLS)
        dstW = dst_pad.reshape(GPC, COLS, 16).transpose(0, 2, 1) \
            .reshape(128, COLS).astype(np.int16)
        in_maps.append({
            "x": np.ascontiguousarray(x[c * NPC:(c + 1) * NPC],
                                      dtype=np.float32),
            "hiH": (srcH >> 5).astype(ml_dtypes.bfloat16),
            "loH": (srcH & 31).astype(ml_dtypes.bfloat16),
            "dstW": dstW,
            "bases": np.array([[c * NPC, c * GPC * K]], np.int32),
        })
        eorders.append(eorder)
    return in_maps, eorders


_CACHE = {}


def _get_nc():
    if "nc" in _CACHE:
        return _CACHE["nc"]
    nc = bacc.Bacc("TRN2", target_bir_lowering=False, debug=False,
                   num_devices=NC)
    ins = {
        "x": nc.dram_tensor("x", (NPC, F), F32, kind="ExternalInput").ap(),
        "hiH": nc.dram_tensor("hiH", (128, COLS), BF16,
                              kind="ExternalInput").ap(),
        "loH": nc.dram_tensor("loH", (128, COLS), BF16,
                              kind="ExternalInput").ap(),
        "dstW": nc.dram_tensor("dstW", (128, COLS), I16,
                               kind="ExternalInput").ap(),
        "bases": nc.dram_tensor("bases", (1, 2), I32,
                                kind="ExternalInput").ap(),
    }
    outs = {
        "xf": nc.dram_tensor("xf", (GPC * K, F), F32,
                             kind="ExternalOutput").ap(),
        "permscore": nc.dram_tensor("permscore", (GPC * K, 2), I32,
                                    kind="ExternalOutput").ap(),
        "rs": nc.dram_tensor("rs", (EC,), I32, kind="ExternalOutput").ap(),
        "rd": nc.dram_tensor("rd", (EC,), I32, kind="ExternalOutput").ap(),
        "mask": nc.dram_tensor("mask", (EC,), U8,
                               kind="ExternalOutput").ap(),
    }
    with tile.TileContext(nc) as tc:
        build_kernel(tc, outs, ins)
    nc.compile()
    _CACHE["nc"] = nc
    return nc


def run_cores(in_maps, trace=False, **kw):
    nc = _get_nc()
    return bass_utils.run_bass_kernel_spmd(
        nc, in_maps, core_ids=list(range(NC)), trace=trace, **kw)


def host_unshard(results, eorders):
    xf = np.concatenate([r["xf"] for r in results], 0)
    perm = np.concatenate([r["permscore"][:, 0] for r in results]) \
        .astype(np.int32)
    score = np.concatenate([r["permscore"][:, 1] for r in results]) \
        .view(np.float32)
    batchf = np.repeat(np.arange(G, dtype=np.int32), K)
    rs = np.empty(E, np.int32)
    rd = np.empty(E, np.int32)
    msk = np.zeros(E, bool)
    for r, eo in zip(results, eorders):
        flat = eo.reshape(-1)
        real = flat >= 0
        rs[flat[real]] = r["rs"][real]
        rd[flat[real]] = r["rd"][real]
        msk[flat[real]] = r["mask"][real].astype(bool)
    remapped = np.stack([rs, rd])
    return (xf, remapped, batchf, perm, score, msk)


def kernel(x, edge_index, batch):
    x = np.asarray(x)
    edge_index = np.asarray(edge_index)
    batch = np.asarray(batch)
    in_maps, eorders = host_shard(x, edge_index, batch)
    res = run_cores(in_maps)
    return host_unshard(res.results, eorders)
